# revision 40
# baseline (speedup 1.0000x reference)
"""Trainium2 Bass kernel for nn_Model_47107201302874.

loss = sum((phi - lam)**2) with phi = kron(v_0..v_25), v_i = [sin|th_i|, cos|th_i|].

Sharding: core d owns the 2^23 lam elements whose top-3 bits equal d.
Locally  phi[p,k,s] = c_d * A[p] * B1[k] * B2[s]  with
  c_d = v_0[b0] v_1[b1] v_2[b2]          (d = b0 b1 b2)
  A   = kron(v_3..v_9)    [128]   (p = bits 3..9)
  B1  = kron(v_10..v_16)  [128]   (k = bits 10..16)
  B2  = kron(v_17..v_25)  [512]   (s = bits 17..25)
Per p-tile [128(k), 512(s)]:
  diff = C * A'[p] - lam_p          (one DVE scalar_tensor_tensor; C = outer(B1,B2))
  acc[:, p] = sum(diff^2) along s   (one ACT Square with accum_out)
Then a free-dim reduce + host gather of the 8x128 partials.
"""

import os
import sys
from contextlib import ExitStack

import numpy as np

for _p in (
    "/opt/trn_rl_repo",
    "/root/.axon_site/_ro/trn_rl_repo",
    "/root/.axon_site/_ro/pypackages",
):
    if os.path.isdir(_p) and _p not in sys.path:
        sys.path.append(_p)

import concourse.bacc as bacc
import concourse.mybir as mybir
import concourse.tile as tile
from concourse.bass_utils import run_bass_kernel_spmd

F32 = mybir.dt.float32
BF16 = mybir.dt.bfloat16
ALU = mybir.AluOpType
ACTF = mybir.ActivationFunctionType

N = 26
NCORES = 8
P, K, S = 128, 128, 512  # p: bits 3..9, k: bits 10..16, s: bits 17..25
# chunk size 2 (512 KiB DMAs): small steady-state compute lag behind the DMA
# stream while keeping per-op fixed costs amortized.
CHUNKS = [2] * 63 + [1, 1]
assert sum(CHUNKS) == P
LAM_BUFS = 16
USE_CUSTOM_DVE = True
# loss = sum(lam^2) - 2*sum(phi*lam) + sum(phi^2): ACT squares lam, PE does
# the cross term as PSUM-accumulated matmuls, DVE is nearly idle.
USE_MATMUL = False
# hybrid: even p-subtiles use the fused DVE op; odd ones below PE_LAST use
# the bf16-PE cross-term path (DVE only casts to bf16 at 2x mode). Spreads
# the compute across DVE/PE/ACT so no engine approaches the DMA floor, and
# ends the PE stream early so its epilogue overlaps the final DMA chunks.
USE_HYBRID = True
PE_LAST = 112
# fraction of sub-tiles diffed on GPSIMD (+ACT square) instead of the DVE
# custom op: 0 = all DVE. With both streams the per-engine busy drops well
# under the DMA floor. 2 = every 2nd sub-tile on POOL.
POOL_EVERY = 0

PI = float(np.pi)

_CACHE = {}


def _register_sqdiff_op():
    """Register a fused DVE op: out = (in0*s0 - in1)^2, accum_out = sum(out).

    One DVE pass replaces the scalar_tensor_tensor + ACT Square pair, so the
    whole reduction runs on the vector engine with no activation stage.
    """
    from operator import add

    from concourse import dve_ops, dve_spec
    from concourse.dve_uop import DveOpSpec

    name = "SQDIFF_ACC_ANT"
    for op in dve_ops.OPS:
        if op.name == name:
            return op

    def ref(in0, in1, c0, c1, c2):
        b = ((in0.astype(np.float32) * c0 - in1) ** 2).astype(np.float32)
        return b, b.reshape(b.shape[0], -1).sum(axis=-1, keepdims=True)

    spec = dve_spec.Spec(
        body=dve_spec.sq(dve_spec.Src0 * dve_spec.C0 - dve_spec.Src1),
        accum=add,
        accum_init=dve_spec.Zero,
        reference=ref,
    )
    row = dve_ops._CUSTOM_DVE_ROW_BASE + len(dve_ops.OPS)
    assert row < 0x20, "custom-DVE opcode rows exhausted"
    dve_ops._SUB_OPCODE_FOR_NAME[name] = row
    shas = {}
    for ver in ("v3", "v4"):
        uops = dve_spec.lower(spec, ver=ver)
        shas[ver] = DveOpSpec(
            name=name, opcode=row, uops=uops, rd1_en=dve_spec._has_src1(spec)
        ).sha(ver)
    op = dve_ops.DveOp(name, spec, subdim=False, uops_sha=shas)
    dve_ops.OPS.append(op)
    dve_ops.CUSTOM_DVE_SPECS[name] = spec
    return op


def _body(ctx, tc, out_ap, theta_ap, dbits_ap, lam_ap, reps=1, loop=False):
    nc = tc.nc
    const = ctx.enter_context(tc.tile_pool(name="const", bufs=1))
    psum = ctx.enter_context(tc.tile_pool(name="psum", bufs=1, space="PSUM"))
    lam_pool = ctx.enter_context(tc.tile_pool(name="lam", bufs=LAM_BUFS))
    scratch = ctx.enter_context(tc.tile_pool(name="scratch", bufs=3))

    # ---- prologue: per-factor sin/cos ------------------------------------
    # gpsimd (SWDGE) for the tiny loads keeps the sync HWDGE queue free to
    # start streaming lam immediately.
    th = const.tile([1, N], F32, tag="th")
    nc.gpsimd.dma_start(th[:], theta_ap)
    db = const.tile([1, 3], F32, tag="db")
    nc.gpsimd.dma_start(db[:], dbits_ap)

    av = const.tile([1, N], F32, tag="av")
    nc.scalar.activation(av[:], th[:], ACTF.Abs)

    # Sin LUT only valid on [-pi, pi]: wrap x (in [0, 3pi)) to x - 2pi*(x > pi).
    sn = const.tile([1, N], F32, tag="sn")
    cs = const.tile([1, N], F32, tag="cs")
    wa = const.tile([1, N], F32, tag="wa")
    wm = const.tile([1, N], F32, tag="wm")
    for dst, shift in ((sn, 0.0), (cs, PI / 2)):
        # wa = |th| + shift ; wm = (wa > pi) ; wa -= 2pi*wm ; dst = Sin(wa)
        if shift:
            nc.vector.tensor_scalar_add(wa[:], av[:], shift)
        else:
            nc.vector.tensor_copy(wa[:], av[:])
        nc.vector.tensor_scalar(wm[:], wa[:], PI, None, op0=ALU.is_gt)
        nc.vector.scalar_tensor_tensor(
            wa[:], wm[:], -2.0 * PI, wa[:], op0=ALU.mult, op1=ALU.add
        )
        nc.scalar.activation(dst[:], wa[:], ACTF.Sin)

    # c_d = prod_i (sn[i] + dbits[i]*(cs[i]-sn[i])), i<3
    sel = const.tile([1, 3], F32, tag="sel")
    nc.vector.tensor_sub(sel[:], cs[0:1, 0:3], sn[0:1, 0:3])
    nc.vector.tensor_mul(sel[:], sel[:], db[:])
    nc.vector.tensor_add(sel[:], sel[:], sn[0:1, 0:3])
    cd = const.tile([1, 1], F32, tag="cd")
    nc.vector.tensor_mul(cd[:], sel[0:1, 0:1], sel[0:1, 1:2])
    nc.vector.tensor_mul(cd[:], cd[:], sel[0:1, 2:3])

    # ---- kron ladders (free dim of partition 0) --------------------------
    kr_a = const.tile([1, S], F32, tag="kr_a")
    kr_b = const.tile([1, S], F32, tag="kr_b")

    kr_c = const.tile([1, S], F32, tag="kr_c")
    kr_d = const.tile([1, S], F32, tag="kr_d")

    def kron(idxs, seed, bufs, eng):
        # ladder on `eng`: DVE uses tensor_scalar_mul, ACT uses Copy+scale —
        # splitting the chains across engines shortens the DVE prologue.
        cur, other = bufs

        def mul(dst, src, L, sc):
            if eng == "act":
                nc.scalar.activation(dst, src[0:1, 0:L], ACTF.Copy, scale=sc)
            else:
                nc.vector.tensor_scalar_mul(dst, src[0:1, 0:L], sc)

        if seed is None:
            nc.vector.memset(cur[0:1, 0:1], 1.0)
        else:
            nc.vector.tensor_copy(cur[0:1, 0:1], seed)
        L = 1
        for i in idxs:
            d3 = other[0:1, 0 : 2 * L].rearrange("a (l t) -> a l t", t=2)
            mul(d3[:, :, 0], cur, L, sn[0:1, i : i + 1])
            mul(d3[:, :, 1], cur, L, cs[0:1, i : i + 1])
            cur, other = other, cur
            L *= 2
        return cur[0:1, 0:L]

    arow_src = kron(range(3, 10), cd, (kr_a, kr_b), "act")  # [1,128] = c_d*A
    arow = const.tile([1, P], F32, tag="arow")
    nc.vector.tensor_copy(arow[:], arow_src)

    b2row_src = kron(range(17, 26), None, (kr_c, kr_d), "dve")  # [1,512]
    b2row = const.tile([1, S], F32, tag="b2row")
    nc.vector.tensor_copy(b2row[:], b2row_src)
    b1row_src = kron(range(10, 17), None, (kr_c, kr_d), "dve")  # [1,128]
    b1row = const.tile([1, P], F32, tag="b1row")
    nc.vector.tensor_copy(b1row[:], b1row_src)

    ct = arep = None
    if USE_MATMUL or USE_HYBRID:
        # H[k, p] = B1[k] * A'[p]; kept in bf16 — the cross term is O(1)
        # against a ~7e7 loss, so bf16 rounding there is invisible.
        h_ps = psum.tile([P, P], F32, tag="h_ps")
        nc.tensor.matmul(h_ps[:], lhsT=b1row[:], rhs=arow[:], start=True, stop=True)
        hmat = const.tile([P, P], BF16, tag="hmat")
        nc.scalar.copy(hmat[:], h_ps[:])

        # phi2 = sum(phi^2) over the PE-path subtiles:
        # (sum of A'[p]^2 over assigned p) * sum(B1^2) * sum(B2^2)
        phi2 = const.tile([1, 1], F32, tag="phi2")
        p2t = const.tile([1, S], F32, tag="p2t")
        p2s = const.tile([1, 1], F32, tag="p2s")
        nc.vector.memset(phi2[:], 1.0)
        nc.vector.tensor_mul(p2t[0:1, 0:P], arow[:], arow[:])
        asq = p2t[0:1, 1:PE_LAST:2] if USE_HYBRID else p2t[0:1, 0:P]
        nc.vector.tensor_reduce(
            p2s[:], asq, axis=mybir.AxisListType.X, op=ALU.add
        )
        nc.vector.tensor_copy(phi2[:], p2s[:])
        for row, ln in ((b1row, P), (b2row, S)):
            nc.vector.tensor_mul(p2t[0:1, 0:ln], row[0:1, 0:ln], row[0:1, 0:ln])
            nc.vector.tensor_reduce(
                p2s[:], p2t[0:1, 0:ln], axis=mybir.AxisListType.X, op=ALU.add
            )
            nc.vector.tensor_mul(phi2[:], phi2[:], p2s[:])
    if not USE_MATMUL:
        ones_r = const.tile([1, P], F32, tag="ones")
        nc.vector.memset(ones_r[:], 1.0)

        c_ps = psum.tile([P, S], F32, tag="c_ps")
        nc.tensor.matmul(c_ps[:], lhsT=b1row[:], rhs=b2row[:], start=True, stop=True)
        ct = const.tile([P, S], F32, tag="ct")
        nc.scalar.copy(ct[:], c_ps[:])

        a_ps = psum.tile([P, P], F32, tag="a_ps")
        nc.tensor.matmul(a_ps[:], lhsT=ones_r[:], rhs=arow[:], start=True, stop=True)
        arep = const.tile([P, P], F32, tag="arep")
        nc.scalar.copy(arep[:], a_ps[:])

    # ---- main loop -------------------------------------------------------
    use_custom = USE_CUSTOM_DVE and not USE_MATMUL
    sqdiff = _register_sqdiff_op() if use_custom else None
    acc = const.tile([P, P if use_custom else len(CHUNKS)], F32, tag="acc")
    lam_r = lam_ap.rearrange("p k s -> k p s")
    if USE_MATMUL or USE_HYBRID:
        w_ps = psum.tile([1, S], F32, tag="w_ps")

    def main_pass():
        p0 = 0
        for t, cnt in enumerate(CHUNKS):
            lt = lam_pool.tile([K, cnt, S], F32, tag="lt")
            nc.sync.dma_start(lt[:], lam_r[:, p0 : p0 + cnt, :])
            if USE_MATMUL:
                # DVE: bf16 copy of the chunk; PE: w[s] += sum_k H[k,p]*lam[p,k,s]
                ltb = scratch.tile([K, cnt, S], BF16, tag="ltb")
                nc.vector.tensor_copy(
                    ltb[:].rearrange("k a s -> k (a s)"),
                    lt[:].rearrange("k a s -> k (a s)"),
                )
                for j in range(cnt):
                    p = p0 + j
                    nc.tensor.matmul(
                        w_ps[:],
                        lhsT=hmat[:, p : p + 1],
                        rhs=ltb[:, j, :],
                        start=(p == 0),
                        stop=(p == P - 1),
                    )
                # ACT: acc[:, t] = sum(lam^2) for this chunk (to scratch)
                sq = scratch.tile([K, cnt, S], F32, tag="sqout")
                nc.scalar.activation(
                    sq[:].rearrange("k a s -> k (a s)"),
                    lt[:].rearrange("k a s -> k (a s)"),
                    ACTF.Square,
                    accum_out=acc[:, t : t + 1],
                )
            elif use_custom:
                for j in range(cnt):
                    p = p0 + j
                    sl = lt[:, j, :]
                    if USE_HYBRID and p % 2 == 1 and p < PE_LAST:
                        # PE path: bf16 cast (DVE 2x) + cross-term matmul;
                        # ACT squares the fp32 subtile for the lam^2 term.
                        ltb = scratch.tile([K, S], BF16, tag="ltb")
                        nc.vector.tensor_copy(ltb[:], sl)
                        nc.tensor.matmul(
                            w_ps[:],
                            lhsT=hmat[:, p : p + 1],
                            rhs=ltb[:],
                            start=(p == 1),
                            stop=(p == PE_LAST - 1),
                        )
                        sq = scratch.tile([K, S], F32, tag="sqout")
                        nc.scalar.activation(
                            sq[:], sl, ACTF.Square, accum_out=acc[:, p : p + 1]
                        )
                    elif POOL_EVERY and p % POOL_EVERY == (POOL_EVERY - 1):
                        # ACT: phi = C * A'[p]; POOL: sl -= phi; ACT: square+acc
                        phi = scratch.tile([K, S], F32, tag="phi")
                        nc.scalar.activation(
                            phi[:], ct[:], ACTF.Copy, scale=arep[:, p : p + 1]
                        )
                        nc.gpsimd.tensor_tensor(sl, sl, phi[:], op=ALU.subtract)
                        nc.scalar.activation(
                            sl, sl, ACTF.Square, accum_out=acc[:, p : p + 1]
                        )
                    else:
                        nc.vector._custom_dve(
                            sqdiff,
                            out=sl,
                            in0=ct[:],
                            in1=sl,
                            s0=arep[:, p : p + 1],
                            accum_out=acc[:, p : p + 1],
                        )
            else:
                for j in range(cnt):
                    p = p0 + j
                    sl = lt[:, j, :]
                    nc.vector.scalar_tensor_tensor(
                        sl, ct[:], arep[:, p : p + 1], sl,
                        op0=ALU.mult, op1=ALU.subtract,
                    )
                flat = lt[:].rearrange("k a s -> k (a s)")
                nc.scalar.activation(
                    flat, flat, ACTF.Square, accum_out=acc[:, t : t + 1]
                )
            p0 += cnt

    if loop and reps > 1:
        with tc.For_i(0, reps, 1):
            main_pass()
    else:
        for _rep in range(reps):
            main_pass()

    # ---- epilogue --------------------------------------------------------
    rsum = const.tile([P, 1], F32, tag="rsum")
    nc.vector.tensor_reduce(rsum[:], acc[:], axis=mybir.AxisListType.X, op=ALU.add)
    if USE_MATMUL or USE_HYBRID:
        # loss_local = sum(lam^2) - 2*cross + phi2 ; fold scalars into rsum[0]
        wrow = const.tile([1, S], F32, tag="wrow")
        nc.scalar.copy(wrow[:], w_ps[:])
        cm = const.tile([1, S], F32, tag="cm")
        nc.vector.tensor_mul(cm[:], wrow[:], b2row[:])
        cross = const.tile([1, 1], F32, tag="cross")
        nc.vector.tensor_reduce(
            cross[:], cm[:], axis=mybir.AxisListType.X, op=ALU.add
        )
        extra = const.tile([1, 1], F32, tag="extra")
        nc.vector.scalar_tensor_tensor(
            extra[:], cross[:], -2.0, phi2[:], op0=ALU.mult, op1=ALU.add
        )
        nc.vector.tensor_add(rsum[0:1, 0:1], rsum[0:1, 0:1], extra[:])
    nc.sync.dma_start(out_ap, rsum[:])


def build_nc(reps=1, loop=False):
    key = ("nc", reps, loop)
    if key in _CACHE:
        return _CACHE[key]
    nc = bacc.Bacc(
        "TRN2", target_bir_lowering=False, debug=False, num_devices=NCORES
    )
    theta_ap = nc.dram_tensor("theta", [1, N], F32, kind="ExternalInput").ap()
    dbits_ap = nc.dram_tensor("dbits", [1, 3], F32, kind="ExternalInput").ap()
    lam_ap = nc.dram_tensor("lam", [P, K, S], F32, kind="ExternalInput").ap()
    out_ap = nc.dram_tensor("partial", [P, 1], F32, kind="ExternalOutput").ap()
    with tile.TileContext(nc) as tc, ExitStack() as ctx:
        _body(ctx, tc, out_ap, theta_ap, dbits_ap, lam_ap, reps=reps, loop=loop)
    nc.compile()
    _CACHE[key] = nc
    return nc


def make_in_maps(theta, lam):
    theta = np.ascontiguousarray(np.asarray(theta, dtype=np.float32)).reshape(1, N)
    lam = np.ascontiguousarray(np.asarray(lam, dtype=np.float32)).reshape(
        NCORES, P, K, S
    )
    in_maps = []
    for d in range(NCORES):
        bits = np.array(
            [[(d >> 2) & 1, (d >> 1) & 1, d & 1]], dtype=np.float32
        )
        in_maps.append({"theta": theta, "dbits": bits, "lam": lam[d]})
    return in_maps


def run(theta, lam, trace=False, **kwargs):
    nc = build_nc()
    in_maps = make_in_maps(theta, lam)
    res = run_bass_kernel_spmd(
        nc, in_maps, list(range(NCORES)), trace=trace, **kwargs
    )
    total = np.float64(0.0)
    for r in res.results:
        total += r["partial"].astype(np.float64).sum()
    return np.array(np.float32(total)), res


def kernel(theta, lam):
    out, _ = run(theta, lam)
    return out


# revision 41
# speedup vs baseline: 1.0577x; 1.0577x over previous
"""Trainium2 Bass kernel for nn_Model_47107201302874.

loss = sum((phi - lam)**2) with phi = kron(v_0..v_25), v_i = [sin|th_i|, cos|th_i|].

Sharding: core d owns the 2^23 lam elements whose top-3 bits equal d.
Locally  phi[p,k,s] = c_d * A[p] * B1[k] * B2[s]  with
  c_d = v_0[b0] v_1[b1] v_2[b2]          (d = b0 b1 b2)
  A   = kron(v_3..v_9)    [128]   (p = bits 3..9)
  B1  = kron(v_10..v_16)  [128]   (k = bits 10..16)
  B2  = kron(v_17..v_25)  [512]   (s = bits 17..25)
Per p-tile [128(k), 512(s)]:
  diff = C * A'[p] - lam_p          (one DVE scalar_tensor_tensor; C = outer(B1,B2))
  acc[:, p] = sum(diff^2) along s   (one ACT Square with accum_out)
Then a free-dim reduce + host gather of the 8x128 partials.
"""

import os
import sys
from contextlib import ExitStack

import numpy as np

for _p in (
    "/opt/trn_rl_repo",
    "/root/.axon_site/_ro/trn_rl_repo",
    "/root/.axon_site/_ro/pypackages",
):
    if os.path.isdir(_p) and _p not in sys.path:
        sys.path.append(_p)

import concourse.bacc as bacc
import concourse.mybir as mybir
import concourse.tile as tile
from concourse.bass_utils import run_bass_kernel_spmd

F32 = mybir.dt.float32
BF16 = mybir.dt.bfloat16
ALU = mybir.AluOpType
ACTF = mybir.ActivationFunctionType

N = 26
NCORES = 8
P, K, S = 128, 128, 512  # p: bits 3..9, k: bits 10..16, s: bits 17..25
# chunk size 2 (512 KiB DMAs): small steady-state compute lag behind the DMA
# stream while keeping per-op fixed costs amortized.
CHUNKS = [2] * 63 + [1, 1]
assert sum(CHUNKS) == P
LAM_BUFS = 24
USE_CUSTOM_DVE = True
# loss = sum(lam^2) - 2*sum(phi*lam) + sum(phi^2): ACT squares lam, PE does
# the cross term as PSUM-accumulated matmuls, DVE is nearly idle.
USE_MATMUL = False
# hybrid: even p-subtiles use the fused DVE op; odd ones below PE_LAST use
# the bf16-PE cross-term path (DVE only casts to bf16 at 2x mode). Spreads
# the compute across DVE/PE/ACT so no engine approaches the DMA floor, and
# ends the PE stream early so its epilogue overlaps the final DMA chunks.
USE_HYBRID = True
PE_LAST = 112
# fraction of sub-tiles diffed on GPSIMD (+ACT square) instead of the DVE
# custom op: 0 = all DVE. With both streams the per-engine busy drops well
# under the DMA floor. 2 = every 2nd sub-tile on POOL.
POOL_EVERY = 0

PI = float(np.pi)

_CACHE = {}


def _register_sqdiff_op():
    """Register a fused DVE op: out = (in0*s0 - in1)^2, accum_out = sum(out).

    One DVE pass replaces the scalar_tensor_tensor + ACT Square pair, so the
    whole reduction runs on the vector engine with no activation stage.
    """
    from operator import add

    from concourse import dve_ops, dve_spec
    from concourse.dve_uop import DveOpSpec

    name = "SQDIFF_ACC_ANT"
    for op in dve_ops.OPS:
        if op.name == name:
            return op

    def ref(in0, in1, c0, c1, c2):
        b = ((in0.astype(np.float32) * c0 - in1) ** 2).astype(np.float32)
        return b, b.reshape(b.shape[0], -1).sum(axis=-1, keepdims=True)

    spec = dve_spec.Spec(
        body=dve_spec.sq(dve_spec.Src0 * dve_spec.C0 - dve_spec.Src1),
        accum=add,
        accum_init=dve_spec.Zero,
        reference=ref,
    )
    row = dve_ops._CUSTOM_DVE_ROW_BASE + len(dve_ops.OPS)
    assert row < 0x20, "custom-DVE opcode rows exhausted"
    dve_ops._SUB_OPCODE_FOR_NAME[name] = row
    shas = {}
    for ver in ("v3", "v4"):
        uops = dve_spec.lower(spec, ver=ver)
        shas[ver] = DveOpSpec(
            name=name, opcode=row, uops=uops, rd1_en=dve_spec._has_src1(spec)
        ).sha(ver)
    op = dve_ops.DveOp(name, spec, subdim=False, uops_sha=shas)
    dve_ops.OPS.append(op)
    dve_ops.CUSTOM_DVE_SPECS[name] = spec
    return op


def _body(ctx, tc, out_ap, theta_ap, dbits_ap, lam_ap, reps=1, loop=False):
    nc = tc.nc
    const = ctx.enter_context(tc.tile_pool(name="const", bufs=1))
    psum = ctx.enter_context(tc.tile_pool(name="psum", bufs=1, space="PSUM"))
    lam_pool = ctx.enter_context(tc.tile_pool(name="lam", bufs=LAM_BUFS))
    scratch = ctx.enter_context(tc.tile_pool(name="scratch", bufs=3))

    # ---- prologue: per-factor sin/cos ------------------------------------
    # gpsimd (SWDGE) for the tiny loads keeps the sync HWDGE queue free to
    # start streaming lam immediately.
    th = const.tile([1, N], F32, tag="th")
    nc.gpsimd.dma_start(th[:], theta_ap)
    db = const.tile([1, 3], F32, tag="db")
    nc.gpsimd.dma_start(db[:], dbits_ap)

    av = const.tile([1, N], F32, tag="av")
    nc.scalar.activation(av[:], th[:], ACTF.Abs)

    # Sin LUT only valid on [-pi, pi]: wrap x (in [0, 3pi)) to x - 2pi*(x > pi).
    sn = const.tile([1, N], F32, tag="sn")
    cs = const.tile([1, N], F32, tag="cs")
    wa = const.tile([1, N], F32, tag="wa")
    wm = const.tile([1, N], F32, tag="wm")
    for dst, shift in ((sn, 0.0), (cs, PI / 2)):
        # wa = |th| + shift ; wm = (wa > pi) ; wa -= 2pi*wm ; dst = Sin(wa)
        if shift:
            nc.vector.tensor_scalar_add(wa[:], av[:], shift)
        else:
            nc.vector.tensor_copy(wa[:], av[:])
        nc.vector.tensor_scalar(wm[:], wa[:], PI, None, op0=ALU.is_gt)
        nc.vector.scalar_tensor_tensor(
            wa[:], wm[:], -2.0 * PI, wa[:], op0=ALU.mult, op1=ALU.add
        )
        nc.scalar.activation(dst[:], wa[:], ACTF.Sin)

    # c_d = prod_i (sn[i] + dbits[i]*(cs[i]-sn[i])), i<3
    sel = const.tile([1, 3], F32, tag="sel")
    nc.vector.tensor_sub(sel[:], cs[0:1, 0:3], sn[0:1, 0:3])
    nc.vector.tensor_mul(sel[:], sel[:], db[:])
    nc.vector.tensor_add(sel[:], sel[:], sn[0:1, 0:3])
    cd = const.tile([1, 1], F32, tag="cd")
    nc.vector.tensor_mul(cd[:], sel[0:1, 0:1], sel[0:1, 1:2])
    nc.vector.tensor_mul(cd[:], cd[:], sel[0:1, 2:3])

    # ---- kron ladders (free dim of partition 0) --------------------------
    kr_a = const.tile([1, S], F32, tag="kr_a")
    kr_b = const.tile([1, S], F32, tag="kr_b")

    kr_c = const.tile([1, S], F32, tag="kr_c")
    kr_d = const.tile([1, S], F32, tag="kr_d")

    def kron(idxs, seed, bufs, eng):
        # ladder on `eng`: DVE uses tensor_scalar_mul, ACT uses Copy+scale —
        # splitting the chains across engines shortens the DVE prologue.
        cur, other = bufs

        def mul(dst, src, L, sc):
            if eng == "act":
                nc.scalar.activation(dst, src[0:1, 0:L], ACTF.Copy, scale=sc)
            else:
                nc.vector.tensor_scalar_mul(dst, src[0:1, 0:L], sc)

        if seed is None:
            nc.vector.memset(cur[0:1, 0:1], 1.0)
        else:
            nc.vector.tensor_copy(cur[0:1, 0:1], seed)
        L = 1
        for i in idxs:
            d3 = other[0:1, 0 : 2 * L].rearrange("a (l t) -> a l t", t=2)
            mul(d3[:, :, 0], cur, L, sn[0:1, i : i + 1])
            mul(d3[:, :, 1], cur, L, cs[0:1, i : i + 1])
            cur, other = other, cur
            L *= 2
        return cur[0:1, 0:L]

    arow_src = kron(range(3, 10), cd, (kr_a, kr_b), "act")  # [1,128] = c_d*A
    arow = const.tile([1, P], F32, tag="arow")
    nc.vector.tensor_copy(arow[:], arow_src)

    b2row_src = kron(range(17, 26), None, (kr_c, kr_d), "dve")  # [1,512]
    b2row = const.tile([1, S], F32, tag="b2row")
    nc.vector.tensor_copy(b2row[:], b2row_src)
    b1row_src = kron(range(10, 17), None, (kr_c, kr_d), "dve")  # [1,128]
    b1row = const.tile([1, P], F32, tag="b1row")
    nc.vector.tensor_copy(b1row[:], b1row_src)

    ct = arep = None
    if USE_MATMUL or USE_HYBRID:
        # H[k, p] = B1[k] * A'[p]; kept in bf16 — the cross term is O(1)
        # against a ~7e7 loss, so bf16 rounding there is invisible.
        h_ps = psum.tile([P, P], F32, tag="h_ps")
        nc.tensor.matmul(h_ps[:], lhsT=b1row[:], rhs=arow[:], start=True, stop=True)
        hmat = const.tile([P, P], BF16, tag="hmat")
        nc.scalar.copy(hmat[:], h_ps[:])

        # phi2 = sum(phi^2) over the PE-path subtiles:
        # (sum of A'[p]^2 over assigned p) * sum(B1^2) * sum(B2^2)
        phi2 = const.tile([1, 1], F32, tag="phi2")
        p2t = const.tile([1, S], F32, tag="p2t")
        p2s = const.tile([1, 1], F32, tag="p2s")
        nc.vector.memset(phi2[:], 1.0)
        nc.vector.tensor_mul(p2t[0:1, 0:P], arow[:], arow[:])
        asq = p2t[0:1, 1:PE_LAST:2] if USE_HYBRID else p2t[0:1, 0:P]
        nc.vector.tensor_reduce(
            p2s[:], asq, axis=mybir.AxisListType.X, op=ALU.add
        )
        nc.vector.tensor_copy(phi2[:], p2s[:])
        for row, ln in ((b1row, P), (b2row, S)):
            nc.vector.tensor_mul(p2t[0:1, 0:ln], row[0:1, 0:ln], row[0:1, 0:ln])
            nc.vector.tensor_reduce(
                p2s[:], p2t[0:1, 0:ln], axis=mybir.AxisListType.X, op=ALU.add
            )
            nc.vector.tensor_mul(phi2[:], phi2[:], p2s[:])
    if not USE_MATMUL:
        ones_r = const.tile([1, P], F32, tag="ones")
        nc.vector.memset(ones_r[:], 1.0)

        c_ps = psum.tile([P, S], F32, tag="c_ps")
        nc.tensor.matmul(c_ps[:], lhsT=b1row[:], rhs=b2row[:], start=True, stop=True)
        ct = const.tile([P, S], F32, tag="ct")
        nc.scalar.copy(ct[:], c_ps[:])

        a_ps = psum.tile([P, P], F32, tag="a_ps")
        nc.tensor.matmul(a_ps[:], lhsT=ones_r[:], rhs=arow[:], start=True, stop=True)
        arep = const.tile([P, P], F32, tag="arep")
        nc.scalar.copy(arep[:], a_ps[:])

    # ---- main loop -------------------------------------------------------
    use_custom = USE_CUSTOM_DVE and not USE_MATMUL
    sqdiff = _register_sqdiff_op() if use_custom else None
    acc = const.tile([P, P if use_custom else len(CHUNKS)], F32, tag="acc")
    lam_r = lam_ap.rearrange("p k s -> k p s")
    if USE_MATMUL or USE_HYBRID:
        w_ps = psum.tile([1, S], F32, tag="w_ps")

    def main_pass():
        p0 = 0
        for t, cnt in enumerate(CHUNKS):
            lt = lam_pool.tile([K, cnt, S], F32, tag="lt")
            nc.sync.dma_start(lt[:], lam_r[:, p0 : p0 + cnt, :])
            if USE_MATMUL:
                # DVE: bf16 copy of the chunk; PE: w[s] += sum_k H[k,p]*lam[p,k,s]
                ltb = scratch.tile([K, cnt, S], BF16, tag="ltb")
                nc.vector.tensor_copy(
                    ltb[:].rearrange("k a s -> k (a s)"),
                    lt[:].rearrange("k a s -> k (a s)"),
                )
                for j in range(cnt):
                    p = p0 + j
                    nc.tensor.matmul(
                        w_ps[:],
                        lhsT=hmat[:, p : p + 1],
                        rhs=ltb[:, j, :],
                        start=(p == 0),
                        stop=(p == P - 1),
                    )
                # ACT: acc[:, t] = sum(lam^2) for this chunk (to scratch)
                sq = scratch.tile([K, cnt, S], F32, tag="sqout")
                nc.scalar.activation(
                    sq[:].rearrange("k a s -> k (a s)"),
                    lt[:].rearrange("k a s -> k (a s)"),
                    ACTF.Square,
                    accum_out=acc[:, t : t + 1],
                )
            elif use_custom:
                for j in range(cnt):
                    p = p0 + j
                    sl = lt[:, j, :]
                    if USE_HYBRID and p % 2 == 1 and p < PE_LAST:
                        # PE path: bf16 cast (DVE 2x) + cross-term matmul;
                        # ACT squares the fp32 subtile for the lam^2 term.
                        ltb = scratch.tile([K, S], BF16, tag="ltb")
                        nc.vector.tensor_copy(ltb[:], sl)
                        nc.tensor.matmul(
                            w_ps[:],
                            lhsT=hmat[:, p : p + 1],
                            rhs=ltb[:],
                            start=(p == 1),
                            stop=(p == PE_LAST - 1),
                        )
                        sq = scratch.tile([K, S], F32, tag="sqout")
                        nc.scalar.activation(
                            sq[:], sl, ACTF.Square, accum_out=acc[:, p : p + 1]
                        )
                    elif POOL_EVERY and p % POOL_EVERY == (POOL_EVERY - 1):
                        # ACT: phi = C * A'[p]; POOL: sl -= phi; ACT: square+acc
                        phi = scratch.tile([K, S], F32, tag="phi")
                        nc.scalar.activation(
                            phi[:], ct[:], ACTF.Copy, scale=arep[:, p : p + 1]
                        )
                        nc.gpsimd.tensor_tensor(sl, sl, phi[:], op=ALU.subtract)
                        nc.scalar.activation(
                            sl, sl, ACTF.Square, accum_out=acc[:, p : p + 1]
                        )
                    else:
                        nc.vector._custom_dve(
                            sqdiff,
                            out=sl,
                            in0=ct[:],
                            in1=sl,
                            s0=arep[:, p : p + 1],
                            accum_out=acc[:, p : p + 1],
                        )
            else:
                for j in range(cnt):
                    p = p0 + j
                    sl = lt[:, j, :]
                    nc.vector.scalar_tensor_tensor(
                        sl, ct[:], arep[:, p : p + 1], sl,
                        op0=ALU.mult, op1=ALU.subtract,
                    )
                flat = lt[:].rearrange("k a s -> k (a s)")
                nc.scalar.activation(
                    flat, flat, ACTF.Square, accum_out=acc[:, t : t + 1]
                )
            p0 += cnt

    if loop and reps > 1:
        with tc.For_i(0, reps, 1):
            main_pass()
    else:
        for _rep in range(reps):
            main_pass()

    # ---- epilogue --------------------------------------------------------
    rsum = const.tile([P, 1], F32, tag="rsum")
    nc.vector.tensor_reduce(rsum[:], acc[:], axis=mybir.AxisListType.X, op=ALU.add)
    if USE_MATMUL or USE_HYBRID:
        # loss_local = sum(lam^2) - 2*cross + phi2 ; fold scalars into rsum[0]
        wrow = const.tile([1, S], F32, tag="wrow")
        nc.scalar.copy(wrow[:], w_ps[:])
        cm = const.tile([1, S], F32, tag="cm")
        nc.vector.tensor_mul(cm[:], wrow[:], b2row[:])
        cross = const.tile([1, 1], F32, tag="cross")
        nc.vector.tensor_reduce(
            cross[:], cm[:], axis=mybir.AxisListType.X, op=ALU.add
        )
        extra = const.tile([1, 1], F32, tag="extra")
        nc.vector.scalar_tensor_tensor(
            extra[:], cross[:], -2.0, phi2[:], op0=ALU.mult, op1=ALU.add
        )
        nc.vector.tensor_add(rsum[0:1, 0:1], rsum[0:1, 0:1], extra[:])
    nc.sync.dma_start(out_ap, rsum[:])


def build_nc(reps=1, loop=False):
    key = ("nc", reps, loop)
    if key in _CACHE:
        return _CACHE[key]
    nc = bacc.Bacc(
        "TRN2", target_bir_lowering=False, debug=False, num_devices=NCORES
    )
    theta_ap = nc.dram_tensor("theta", [1, N], F32, kind="ExternalInput").ap()
    dbits_ap = nc.dram_tensor("dbits", [1, 3], F32, kind="ExternalInput").ap()
    lam_ap = nc.dram_tensor("lam", [P, K, S], F32, kind="ExternalInput").ap()
    out_ap = nc.dram_tensor("partial", [P, 1], F32, kind="ExternalOutput").ap()
    with tile.TileContext(nc) as tc, ExitStack() as ctx:
        _body(ctx, tc, out_ap, theta_ap, dbits_ap, lam_ap, reps=reps, loop=loop)
    nc.compile()
    _CACHE[key] = nc
    return nc


def make_in_maps(theta, lam):
    theta = np.ascontiguousarray(np.asarray(theta, dtype=np.float32)).reshape(1, N)
    lam = np.ascontiguousarray(np.asarray(lam, dtype=np.float32)).reshape(
        NCORES, P, K, S
    )
    in_maps = []
    for d in range(NCORES):
        bits = np.array(
            [[(d >> 2) & 1, (d >> 1) & 1, d & 1]], dtype=np.float32
        )
        in_maps.append({"theta": theta, "dbits": bits, "lam": lam[d]})
    return in_maps


def run(theta, lam, trace=False, **kwargs):
    nc = build_nc()
    in_maps = make_in_maps(theta, lam)
    res = run_bass_kernel_spmd(
        nc, in_maps, list(range(NCORES)), trace=trace, **kwargs
    )
    total = np.float64(0.0)
    for r in res.results:
        total += r["partial"].astype(np.float64).sum()
    return np.array(np.float32(total)), res


def kernel(theta, lam):
    out, _ = run(theta, lam)
    return out
